# revision 32
# baseline (speedup 1.0000x reference)
"""Causal self-attention Bass kernel for 8 trn2 NeuronCores.

Problem: B=4, T=2048, D=1024, H=16 causal self-attention (qkv proj + attn + out proj).

Sharding: core c = 2*b + g handles batch b (=c//2) and head-group g (=c%2, 8 heads).

Structure (J-pipelined, chunked collective):
  - x is transposed + bf16-cast on the HOST: xT [D, T] arrives ready for matmuls.
  - V projection per t-quarter: vv[i] [keys=128, 4mp x (65A|65B)] bf16, ones col
    at slot 64 of each 65-block (softmax denominator via the AV matmul).
  - QK projection per T-quarter n: qkT[m] [128 dims, T] bf16 (+bias).
  - Attention per query block J (512 q) x head pair mp: transposed-score flash
    loop over key blocks j<=diag; exp on ACT -> at bf16; causal masks on DVE;
    AV accumulates [65, 512] psum (row 64 = denominator).
  - Normalization per (mp, J): reciprocal_approx_fast on the denominator row,
    partition_broadcast, muls; B-half is normalized pre-shift then DMA'd to
    partitions 64:128.
  - Output projection per J -> rs_in[J] bf16 [512, 1024]; pairwise
    ReduceScatter per J (overlaps attention of J+1); even core gets rows
    [0:256) of each 512-row chunk, odd gets [256:512).
Host reassembles interleaved 256-row chunks per batch.
"""

from contextlib import ExitStack

import ml_dtypes
import numpy as np

import concourse.bass as bass
import concourse.mybir as mybir
import concourse.tile as tile
from concourse import bacc
from concourse.bass_utils import run_bass_kernel_spmd

B, T, D, H = 4, 2048, 1024, 16
HD = D // H  # 64
NCORES = 8
P = 128
f32 = mybir.dt.float32
f32r = mybir.dt.float32r
bf16 = mybir.dt.bfloat16
EXP = mybir.ActivationFunctionType.Exp

_CACHE = {}
LAST_RESULTS = None
_DEBUG_SINK = None


def _dbg(nc, name, ap):
    if _DEBUG_SINK is not None and name in _DEBUG_SINK:
        nc.sync.dma_start(_DEBUG_SINK[name].ap(), ap)


def _emit(nc, tc, xt_d, wqk_d, wv_d, bqk_d, wproj_d, beta_d, out_d):
    with ExitStack() as ctx:
        # ---------------- constants / persistent tiles ----------------
        const = ctx.enter_context(tc.tile_pool(name="const", bufs=1))
        mask_tri = const.tile([P, P], bf16, tag="mask_tri")
        nc.gpsimd.memset(mask_tri[:], 1.0)
        nc.gpsimd.affine_select(
            out=mask_tri[:], in_=mask_tri[:],
            compare_op=mybir.AluOpType.is_ge, fill=0.0,
            base=0, pattern=[[1, P]], channel_multiplier=-1,
        )
        zeros384 = const.tile([P, 384], bf16, tag="zeros384")
        nc.vector.memset(zeros384[:], 0.0)
        bq = [const.tile([P, 1], f32, tag=f"bq{m}", name=f"bq{m}") for m in range(8)]
        beta_b = const.tile([P, D], bf16, tag="beta_b")

        # persistent activations
        xt_pool = ctx.enter_context(tc.tile_pool(name="xt", bufs=1))
        xT = [xt_pool.tile([P, T], bf16, tag=f"xT{k}", name=f"xT{k}") for k in range(8)]
        qkt_pool = ctx.enter_context(tc.tile_pool(name="qkt", bufs=1))
        qkT = [qkt_pool.tile([P, T], bf16, tag=f"qkT{m}", name=f"qkT{m}") for m in range(8)]
        vv_pool = ctx.enter_context(tc.tile_pool(name="vv", bufs=1))
        vv = [vv_pool.tile([P, 520], bf16, tag=f"vv{i}", name=f"vv{i}") for i in range(16)]

        # weights
        wp = ctx.enter_context(tc.tile_pool(name="wts", bufs=1))
        wqk_t = [wp.tile([P, 1024], bf16, tag=f"wqk{k}", name=f"wqk{k}") for k in range(8)]
        wv_t = [wp.tile([P, 512], bf16, tag=f"wv{k}", name=f"wv{k}") for k in range(8)]
        wproj_t = [wp.tile([P, D], bf16, tag=f"wp{hp}", name=f"wp{hp}") for hp in range(4)]

        # ones columns of vv (denominator trick): col 64 of each 65-block
        ones8 = const.tile([P, 8], bf16, tag="ones8")
        nc.vector.memset(ones8[:], 1.0)
        ones_src = ones8[:].rearrange("p (mp h one) -> p mp h one", mp=4, h=2)
        for i in range(16):
            dst = vv[i][:].rearrange("p (mp h d) -> p mp h d", mp=4, h=2)
            nc.vector.tensor_copy(dst[:, :, :, 64:65], ones_src[:, :, :, :])

        # working pools
        onp = ctx.enter_context(tc.tile_pool(name="outn", bufs=3))
        atp = ctx.enter_context(tc.tile_pool(name="atp", bufs=4))
        nrm = ctx.enter_context(tc.tile_pool(name="nrm", bufs=2))
        finp = ctx.enter_context(tc.tile_pool(name="finp", bufs=3))

        spp = ctx.enter_context(tc.tile_pool(name="spp", bufs=2, space="PSUM"))
        stps = ctx.enter_context(tc.tile_pool(name="stps", bufs=2, space="PSUM"))
        oups = ctx.enter_context(tc.tile_pool(name="oups", bufs=1, space="PSUM"))

        dram = ctx.enter_context(tc.tile_pool(name="dram", bufs=1, space="DRAM"))
        # one RS chunk per 256-row half of each query block J (8 total):
        # separate tiles so each collective depends only on its own writes
        rs_in = [dram.tile([256, D], bf16, tag=f"rsi{c}", name=f"rsi{c}") for c in range(8)]
        rs_out = [dram.tile([128, D], bf16, tag=f"rso{c}", name=f"rso{c}") for c in range(8)]

        # ~4us of dummy matmuls at the head of the PE queue: flips the HAM
        # clock gate to 8/8 while the first input DMAs are still in flight,
        # so the real matmuls start at 2.4 GHz.
        wps = spp.tile([P, 384], f32, tag="sp", name="warm")
        for d in range(24):
            nc.tensor.matmul(
                wps[:], zeros384[:, 0:128], zeros384[:],
                start=(d == 0), stop=(d == 23),
            )

        # ---------------- DMAs: quarter 0 + weights first, 3 queues ----------------
        for k in range(8):
            nc.sync.dma_start(xT[k][:, 0:512], xt_d.ap()[k * P : (k + 1) * P, 0:512])
        for k in range(8):
            nc.scalar.dma_start(wv_t[k][:], wv_d.ap()[k * P : (k + 1) * P, :])
        for k in range(8):
            nc.gpsimd.dma_start(wqk_t[k][:], wqk_d.ap()[k * P : (k + 1) * P, :])
        for m in range(8):
            nc.gpsimd.dma_start(bq[m][:], bqk_d.ap()[m])
        for q in range(1, 4):
            for k in range(8):
                nc.sync.dma_start(
                    xT[k][:, q * 512 : (q + 1) * 512],
                    xt_d.ap()[k * P : (k + 1) * P, q * 512 : (q + 1) * 512],
                )
        for hp in range(4):
            nc.gpsimd.dma_start(wproj_t[hp][:], wproj_d.ap()[hp * P : (hp + 1) * P, :])
        nc.gpsimd.dma_start(beta_b[0:1, :], beta_d.ap())
        nc.gpsimd.partition_broadcast(beta_b[:], beta_b[0:1, :], channels=P)

        def vproj(q, ils=range(4)):
            # v for t-tiles of quarter q: vv[i] [keys=128, (mp h 65)]
            for il in ils:
                i = q * 4 + il
                ps = spp.tile([P, 512], f32, tag="sp", name=f"vp{i}")
                for k in range(8):
                    nc.tensor.matmul(
                        ps[:],
                        xT[k][:, i * P : (i + 1) * P],
                        wv_t[k][:],
                        start=(k == 0), stop=(k == 7),
                    )
                src = ps[:].rearrange("p (mp h d) -> p mp h d", mp=4, h=2)
                dst = vv[i][:].rearrange("p (mp h d) -> p mp h d", mp=4, h=2)
                nc.vector.tensor_copy(dst[:, :, :, 0:64], src[:, :, :, :])

        def qkproj(n, ms=range(8)):
            ns = slice(n * 512, (n + 1) * 512)
            for m in ms:
                ps = spp.tile([P, 512], f32, tag="sp", name=f"qkp{m}n{n}")
                for k in range(8):
                    nc.tensor.matmul(
                        ps[:],
                        wqk_t[k][:, m * P : (m + 1) * P],
                        xT[k][:, ns],
                        start=(k == 0), stop=(k == 7),
                    )
                nc.vector.tensor_scalar_add(qkT[m][:, ns], ps[:], bq[m][:])

        def attn(Q0, W, fillers=(), fine=False):
            # attention for queries [Q0, Q0+W) across all 4 head pairs.
            # W in {256, 512}; key blocks 0..nj-1 cover [0, Q0+W).
            nj = (Q0 + W) // P
            jq = Q0 // P  # first diagonal-straddling key block
            fillers = list(fillers)
            outN = [onp.tile([P, W], bf16, tag=f"outN{mp}", name=f"outN{mp}Q{Q0}")
                    for mp in range(4)]
            for mp in range(4):
                qs, ks = qkT[mp], qkT[4 + mp]
                ouA = oups.tile([65, W], f32, tag="ouA")
                ouB = oups.tile([65, W], f32, tag="ouB")
                for j in range(nj):
                    # the two head halves always sit at column offsets 0 and
                    # 512 (separate PSUM banks — the concurrent row-group
                    # matmuls must not share a bank)
                    sT = stps.tile([P, 1024], f32, tag="sT")
                    js = slice(j * P, (j + 1) * P)
                    i = j - jq
                    # diagonal-straddling blocks: queries < c0 are fully
                    # masked, so scores/exp/AV all restrict to [c0:W)
                    c0 = P * i if i > 0 else 0
                    qJs = slice(Q0 + c0, Q0 + W)
                    w = W - c0
                    nc.tensor.matmul(
                        sT[:, 0:w], ks[0:64, js], qs[0:64, qJs],
                        start=True, stop=True, tile_position=(0, 0),
                    )
                    nc.tensor.matmul(
                        sT[:, 512 : 512 + w], ks[64:128, js], qs[64:128, qJs],
                        start=True, stop=True, tile_position=(64, 0),
                    )
                    at = atp.tile([P, 1024], bf16, tag="at")
                    src_v = sT[:].rearrange("p (h c) -> p h c", h=2)
                    dst_v = at[:].rearrange("p (h c) -> p h c", h=2)
                    nc.scalar.activation(
                        dst_v[:, :, 0:w], src_v[:, :, 0:w],
                        EXP, bias=0.0, scale=0.125,
                    )
                    if i >= 0:
                        # triangle mask on the first live 128 columns
                        for h0 in (0, 512):
                            nc.vector.tensor_mul(
                                at[:, h0 : h0 + P],
                                at[:, h0 : h0 + P], mask_tri[:],
                            )
                    nc.tensor.matmul(
                        ouA[:, c0:W], vv[j][:, 130 * mp : 130 * mp + 65],
                        at[:, 0:w],
                        start=(j == 0), stop=(j == nj - 1),
                    )
                    nc.tensor.matmul(
                        ouB[:, c0:W], vv[j][:, 130 * mp + 65 : 130 * mp + 130],
                        at[:, 512 : 512 + w],
                        start=(j == 0), stop=(j == nj - 1),
                    )
                    if fine and fillers:
                        fillers.pop(0)()
                # normalization: denominator (psum row 64) reciprocal,
                # broadcast, scale. B half normalized at partitions 0:64 then
                # DMA-shifted to outN partitions 64:128.
                dA = nrm.tile([1, W], f32, tag="dA")
                dB = nrm.tile([1, W], f32, tag="dB")
                nc.any.tensor_copy(dA[:], ouA[64:65, :])
                nc.any.tensor_copy(dB[:], ouB[64:65, :])
                dRA = nrm.tile([1, W], f32, tag="dRA")
                dRB = nrm.tile([1, W], f32, tag="dRB")
                nc.vector.reciprocal_approx_fast(dRA[:], dA[:])
                nc.vector.reciprocal_approx_fast(dRB[:], dB[:])
                cA = nrm.tile([1, W], bf16, tag="cA")
                cB = nrm.tile([1, W], bf16, tag="cB")
                nc.vector.tensor_copy(cA[:], dRA[:])
                nc.vector.tensor_copy(cB[:], dRB[:])
                bcA = nrm.tile([64, W], bf16, tag="bcA")
                bcB = nrm.tile([P, W], bf16, tag="bcB")
                nc.gpsimd.partition_broadcast(bcA[:, :], cA[:], channels=64)
                nc.gpsimd.partition_broadcast(bcB[:, :], cB[:], channels=P)
                # evict psum promptly (frees the AV banks for the next head
                # pair); B half is DMA'd raw to partitions 64:128 early and
                # normalized in place (keeps the shift DMA off the chain tail)
                tbA = nrm.tile([64, W], bf16, tag="tbA")
                tbB = nrm.tile([64, W], bf16, tag="tbB")
                nc.any.tensor_copy(tbA[:], ouA[0:64, :])
                nc.any.tensor_copy(tbB[:], ouB[0:64, :])
                nc.sync.dma_start(outN[mp][64:128, :], tbB[:])
                nc.vector.tensor_mul(outN[mp][0:64, :], tbA[:], bcA[:, :])
                nc.vector.tensor_mul(
                    outN[mp][64:128, :], outN[mp][64:128, :], bcB[64:128, :]
                )
                # interleave independent fill work (prev proj / next
                # projections) so the in-order PE queue never starves on the
                # normalization chain or exp latency
                if not fine and fillers:
                    fillers.pop(0)()
            while fillers:
                fillers.pop(0)()
            return outN

        def rs_chunk(c):
            if globals().get("_NO_COLLECTIVE"):
                nc.sync.dma_start(out_d.ap()[c * P : (c + 1) * P, :], rs_in[c][0:P, :])
            else:
                nc.gpsimd.collective_compute(
                    "ReduceScatter", mybir.AluOpType.add,
                    replica_groups=[[0, 1], [2, 3], [4, 5], [6, 7]],
                    ins=[rs_in[c].opt()], outs=[rs_out[c].opt()],
                )
                nc.sync.dma_start(out_d.ap()[c * P : (c + 1) * P, :], rs_out[c][:])

        def proj(Q0, W, outN):
            for i2 in range(W // P):
                c = Q0 // 256 + i2 // 2  # 256-row RS chunk index
                r = (Q0 // P + i2) % 2   # 128-row position within the chunk
                for n in range(2):
                    ps = spp.tile([P, 512], f32, tag="sp", name=f"fpq{Q0}i{i2}n{n}")
                    for hp in range(4):
                        nc.tensor.matmul(
                            ps[:],
                            outN[hp][:, i2 * P : (i2 + 1) * P],
                            wproj_t[hp][:, n * 512 : (n + 1) * 512],
                            start=(hp == 0), stop=(hp == 3),
                        )
                    fin = finp.tile([P, 512], bf16, tag="fin")
                    nc.vector.tensor_add(fin[:], ps[:], beta_b[:, n * 512 : (n + 1) * 512])
                    nc.sync.dma_start(
                        rs_in[c][r * P : (r + 1) * P, n * 512 : (n + 1) * 512],
                        fin[:],
                    )
                if r == 1:
                    rs_chunk(c)

        # ---------------- main pipeline ----------------
        vproj(0)
        qkproj(0)
        pending = None
        blocks = [(0, 512), (512, 512), (1024, 512), (1536, 256), (1792, 256)]
        for bi, (Q0, W) in enumerate(blocks):
            fillers = []
            if pending is not None:
                Qp, Wp, outNp = pending
                fillers.append(lambda Qp=Qp, Wp=Wp, o=outNp: proj(Qp, Wp, o))
            if Q0 == 0:
                # fine-grained fillers: consumed per key-block in attn(0),
                # which is short and exp-latency-bound
                for il in range(4):
                    fillers.append(lambda il=il: vproj(1, [il]))
                for m in range(8):
                    fillers.append(lambda m=m: qkproj(1, [m]))
            elif bi < 3:
                fillers.append(lambda q=bi + 1: vproj(q))
                fillers.append(lambda q=bi + 1: qkproj(q, range(4)))
                fillers.append(lambda q=bi + 1: qkproj(q, range(4, 8)))
            outN = attn(Q0, W, fillers, fine=(Q0 == 0))
            pending = (Q0, W, outN)
        proj(pending[0], pending[1], pending[2])
        _dbg(nc, "qkT0", qkT[0][:])
        _dbg(nc, "qkT4", qkT[4][:])
        _dbg(nc, "vv0", vv[0][:])
        _dbg(nc, "xT0", xT[0][:])


def _build():
    if "nc" in _CACHE:
        return _CACHE["nc"]
    nc = bacc.Bacc("TRN2", target_bir_lowering=False, debug=False, num_devices=NCORES)
    xt_d = nc.dram_tensor("x_t", [D, T], bf16, kind="ExternalInput")
    wqk_d = nc.dram_tensor("w_qk", [D, 1024], bf16, kind="ExternalInput")
    wv_d = nc.dram_tensor("w_v", [D, 512], bf16, kind="ExternalInput")
    bqk_d = nc.dram_tensor("b_qk", [8, P, 1], f32, kind="ExternalInput")
    wproj_d = nc.dram_tensor("w_proj", [512, D], bf16, kind="ExternalInput")
    beta_d = nc.dram_tensor("beta", [1, D], bf16, kind="ExternalInput")
    out_d = nc.dram_tensor("out", [T // 2, D], bf16, kind="ExternalOutput")
    with tile.TileContext(nc) as tc:
        _emit(nc, tc, xt_d, wqk_d, wv_d, bqk_d, wproj_d, beta_d, out_d)
    nc.compile()
    _CACHE["nc"] = nc
    return nc


def make_in_maps(x, w_qkv, b_qkv, w_proj, b_proj):
    x = np.asarray(x, np.float32)
    w_qkv = np.asarray(w_qkv, np.float32)
    b_qkv = np.asarray(b_qkv, np.float32)
    w_proj = np.asarray(w_proj, np.float32)
    b_proj = np.asarray(b_proj, np.float32)
    in_maps = []
    for c in range(NCORES):
        b, g = c // 2, c % 2
        qcols = slice(g * 512, (g + 1) * 512)
        kcols = slice(D + g * 512, D + (g + 1) * 512)
        vcols = slice(2 * D + g * 512, 2 * D + (g + 1) * 512)
        w_qk = np.concatenate([w_qkv[:, qcols], w_qkv[:, kcols]], axis=1)
        b_qk = np.concatenate([b_qkv[qcols], b_qkv[kcols]])
        wp = np.ascontiguousarray(w_proj[g * 512 : (g + 1) * 512, :])
        beta = wp.T @ b_qkv[vcols]
        if g == 0:
            beta = beta + b_proj
        in_maps.append({
            "x_t": np.ascontiguousarray(x[b].T).astype(ml_dtypes.bfloat16),
            "w_qk": np.ascontiguousarray(w_qk).astype(ml_dtypes.bfloat16),
            "w_v": np.ascontiguousarray(w_qkv[:, vcols]).astype(ml_dtypes.bfloat16),
            "b_qk": b_qk.reshape(8, P, 1),
            "w_proj": wp.astype(ml_dtypes.bfloat16),
            "beta": beta.reshape(1, D).astype(ml_dtypes.bfloat16),
        })
    return in_maps


def kernel(x, w_qkv, b_qkv, w_proj, b_proj, trace=False, **run_kwargs):
    global LAST_RESULTS
    nc = _build()
    in_maps = make_in_maps(x, w_qkv, b_qkv, w_proj, b_proj)
    res = run_bass_kernel_spmd(
        nc, in_maps, core_ids=list(range(NCORES)), trace=trace, **run_kwargs
    )
    LAST_RESULTS = res
    out = np.empty((B, T, D), np.float32)
    for b in range(B):
        ev = np.asarray(res.results[2 * b]["out"], dtype=np.float32)
        od = np.asarray(res.results[2 * b + 1]["out"], dtype=np.float32)
        for c in range(8):
            out[b, c * 256 : c * 256 + 128] = ev[c * 128 : (c + 1) * 128]
            out[b, c * 256 + 128 : (c + 1) * 256] = od[c * 128 : (c + 1) * 128]
    return out


# revision 37
# speedup vs baseline: 1.0411x; 1.0411x over previous
"""Causal self-attention Bass kernel for 8 trn2 NeuronCores.

Problem: B=4, T=2048, D=1024, H=16 causal self-attention (qkv proj + attn + out proj).

Sharding: core c = 2*b + g handles batch b (=c//2) and head-group g (=c%2, 8 heads).

Structure (J-pipelined, chunked collective):
  - x is transposed + bf16-cast on the HOST: xT [D, T] arrives ready for matmuls.
  - V projection per t-quarter: vv[i] [keys=128, 4mp x (65A|65B)] bf16, ones col
    at slot 64 of each 65-block (softmax denominator via the AV matmul).
  - QK projection per T-quarter n: qkT[m] [128 dims, T] bf16 (+bias).
  - Attention per query block J (512 q) x head pair mp: transposed-score flash
    loop over key blocks j<=diag; exp on ACT -> at bf16; causal masks on DVE;
    AV accumulates [65, 512] psum (row 64 = denominator).
  - Normalization per (mp, J): reciprocal_approx_fast on the denominator row,
    partition_broadcast, muls; B-half is normalized pre-shift then DMA'd to
    partitions 64:128.
  - Output projection per J -> rs_in[J] bf16 [512, 1024]; pairwise
    ReduceScatter per J (overlaps attention of J+1); even core gets rows
    [0:256) of each 512-row chunk, odd gets [256:512).
Host reassembles interleaved 256-row chunks per batch.
"""

from contextlib import ExitStack

import ml_dtypes
import numpy as np

import concourse.bass as bass
import concourse.mybir as mybir
import concourse.tile as tile
from concourse import bacc
from concourse.bass_utils import run_bass_kernel_spmd

B, T, D, H = 4, 2048, 1024, 16
HD = D // H  # 64
NCORES = 8
P = 128
f32 = mybir.dt.float32
f32r = mybir.dt.float32r
bf16 = mybir.dt.bfloat16
EXP = mybir.ActivationFunctionType.Exp

_CACHE = {}
LAST_RESULTS = None
_DEBUG_SINK = None


def _dbg(nc, name, ap):
    if _DEBUG_SINK is not None and name in _DEBUG_SINK:
        nc.sync.dma_start(_DEBUG_SINK[name].ap(), ap)


def _emit(nc, tc, xt_d, wqk_d, wv_d, bqk_d, wproj_d, beta_d, out_d):
    with ExitStack() as ctx:
        # ---------------- constants / persistent tiles ----------------
        const = ctx.enter_context(tc.tile_pool(name="const", bufs=1))
        mask_tri = const.tile([P, P], bf16, tag="mask_tri")
        nc.gpsimd.memset(mask_tri[:], 1.0)
        nc.gpsimd.affine_select(
            out=mask_tri[:], in_=mask_tri[:],
            compare_op=mybir.AluOpType.is_ge, fill=0.0,
            base=0, pattern=[[1, P]], channel_multiplier=-1,
        )
        zeros384 = const.tile([P, 384], bf16, tag="zeros384")
        nc.vector.memset(zeros384[:], 0.0)
        bq = [const.tile([P, 1], f32, tag=f"bq{m}", name=f"bq{m}") for m in range(8)]
        beta_b = const.tile([P, D], bf16, tag="beta_b")

        # persistent activations
        xt_pool = ctx.enter_context(tc.tile_pool(name="xt", bufs=1))
        xT = [xt_pool.tile([P, T], bf16, tag=f"xT{k}", name=f"xT{k}") for k in range(8)]
        qkt_pool = ctx.enter_context(tc.tile_pool(name="qkt", bufs=1))
        qkT = [qkt_pool.tile([P, T], bf16, tag=f"qkT{m}", name=f"qkT{m}") for m in range(8)]
        vv_pool = ctx.enter_context(tc.tile_pool(name="vv", bufs=1))
        vv = [vv_pool.tile([P, 520], bf16, tag=f"vv{i}", name=f"vv{i}") for i in range(16)]

        # weights
        wp = ctx.enter_context(tc.tile_pool(name="wts", bufs=1))
        wqk_t = [wp.tile([P, 1024], bf16, tag=f"wqk{k}", name=f"wqk{k}") for k in range(8)]
        wv_t = [wp.tile([P, 512], bf16, tag=f"wv{k}", name=f"wv{k}") for k in range(8)]
        wproj_t = [wp.tile([P, D], bf16, tag=f"wp{hp}", name=f"wp{hp}") for hp in range(4)]

        # ones columns of vv (denominator trick): col 64 of each 65-block
        ones8 = const.tile([P, 8], bf16, tag="ones8")
        nc.vector.memset(ones8[:], 1.0)
        ones_src = ones8[:].rearrange("p (mp h one) -> p mp h one", mp=4, h=2)
        for i in range(16):
            dst = vv[i][:].rearrange("p (mp h d) -> p mp h d", mp=4, h=2)
            nc.vector.tensor_copy(dst[:, :, :, 64:65], ones_src[:, :, :, :])

        # working pools
        onp = ctx.enter_context(tc.tile_pool(name="outn", bufs=3))
        atp = ctx.enter_context(tc.tile_pool(name="atp", bufs=4))
        nrm = ctx.enter_context(tc.tile_pool(name="nrm", bufs=2))
        finp = ctx.enter_context(tc.tile_pool(name="finp", bufs=3))

        spp = ctx.enter_context(tc.tile_pool(name="spp", bufs=2, space="PSUM"))
        stps = ctx.enter_context(tc.tile_pool(name="stps", bufs=2, space="PSUM"))
        oups = ctx.enter_context(tc.tile_pool(name="oups", bufs=1, space="PSUM"))

        dram = ctx.enter_context(tc.tile_pool(name="dram", bufs=1, space="DRAM"))
        # one RS chunk per 256-row half of each query block J (8 total):
        # separate tiles so each collective depends only on its own writes
        rs_in = [dram.tile([256, D], bf16, tag=f"rsi{c}", name=f"rsi{c}") for c in range(8)]
        rs_out = [dram.tile([128, D], bf16, tag=f"rso{c}", name=f"rso{c}") for c in range(8)]

        # ~4us of dummy matmuls at the head of the PE queue: flips the HAM
        # clock gate to 8/8 while the first input DMAs are still in flight,
        # so the real matmuls start at 2.4 GHz.
        wps = spp.tile([P, 384], f32, tag="sp", name="warm")
        for d in range(24):
            nc.tensor.matmul(
                wps[:], zeros384[:, 0:128], zeros384[:],
                start=(d == 0), stop=(d == 23),
            )

        # ---------------- DMAs: quarter 0 + weights first, 3 queues ----------------
        for k in range(8):
            nc.sync.dma_start(xT[k][:, 0:512], xt_d.ap()[k * P : (k + 1) * P, 0:512])
        for k in range(8):
            nc.scalar.dma_start(wv_t[k][:], wv_d.ap()[k * P : (k + 1) * P, :])
        for k in range(8):
            nc.gpsimd.dma_start(wqk_t[k][:], wqk_d.ap()[k * P : (k + 1) * P, :])
        for m in range(8):
            nc.gpsimd.dma_start(bq[m][:], bqk_d.ap()[m])
        for q in range(1, 4):
            for k in range(8):
                nc.sync.dma_start(
                    xT[k][:, q * 512 : (q + 1) * 512],
                    xt_d.ap()[k * P : (k + 1) * P, q * 512 : (q + 1) * 512],
                )
        for hp in range(4):
            nc.gpsimd.dma_start(wproj_t[hp][:], wproj_d.ap()[hp * P : (hp + 1) * P, :])
        nc.gpsimd.dma_start(beta_b[0:1, :], beta_d.ap())
        nc.gpsimd.partition_broadcast(beta_b[:], beta_b[0:1, :], channels=P)

        def vproj(q, ils=range(4)):
            # v for t-tiles of quarter q: vv[i] [keys=128, (mp h 65)]
            for il in ils:
                i = q * 4 + il
                ps = spp.tile([P, 512], f32, tag="sp", name=f"vp{i}")
                for k in range(8):
                    nc.tensor.matmul(
                        ps[:],
                        xT[k][:, i * P : (i + 1) * P],
                        wv_t[k][:],
                        start=(k == 0), stop=(k == 7),
                    )
                src = ps[:].rearrange("p (mp h d) -> p mp h d", mp=4, h=2)
                dst = vv[i][:].rearrange("p (mp h d) -> p mp h d", mp=4, h=2)
                nc.vector.tensor_copy(dst[:, :, :, 0:64], src[:, :, :, :])

        def qkproj(n, ms=range(8)):
            ns = slice(n * 512, (n + 1) * 512)
            for m in ms:
                ps = spp.tile([P, 512], f32, tag="sp", name=f"qkp{m}n{n}")
                for k in range(8):
                    nc.tensor.matmul(
                        ps[:],
                        wqk_t[k][:, m * P : (m + 1) * P],
                        xT[k][:, ns],
                        start=(k == 0), stop=(k == 7),
                    )
                nc.vector.tensor_scalar_add(qkT[m][:, ns], ps[:], bq[m][:])

        def attn(Q0, W, fillers=()):
            # attention for queries [Q0, Q0+W) across all 4 head pairs.
            # W in {256, 512}; key blocks 0..nj-1 cover [0, Q0+W).
            nj = (Q0 + W) // P
            jq = Q0 // P  # first diagonal-straddling key block
            fillers = list(fillers)
            # spread filler emission evenly over the j-steps so the in-order
            # PE queue has independent matmuls at every exp-latency stall
            step = [0]
            cadence = max(1, (nj * 4) // (len(fillers) + 1)) if fillers else 1 << 30
            outN = [onp.tile([P, W], bf16, tag=f"outN{mp}", name=f"outN{mp}Q{Q0}")
                    for mp in range(4)]
            for mp in range(4):
                qs, ks = qkT[mp], qkT[4 + mp]
                ouA = oups.tile([65, W], f32, tag="ouA")
                ouB = oups.tile([65, W], f32, tag="ouB")
                for j in range(nj):
                    # the two head halves always sit at column offsets 0 and
                    # 512 (separate PSUM banks — the concurrent row-group
                    # matmuls must not share a bank)
                    sT = stps.tile([P, 1024], f32, tag="sT")
                    js = slice(j * P, (j + 1) * P)
                    i = j - jq
                    # diagonal-straddling blocks: queries < c0 are fully
                    # masked, so scores/exp/AV all restrict to [c0:W)
                    c0 = P * i if i > 0 else 0
                    qJs = slice(Q0 + c0, Q0 + W)
                    w = W - c0
                    nc.tensor.matmul(
                        sT[:, 0:w], ks[0:64, js], qs[0:64, qJs],
                        start=True, stop=True, tile_position=(0, 0),
                    )
                    nc.tensor.matmul(
                        sT[:, 512 : 512 + w], ks[64:128, js], qs[64:128, qJs],
                        start=True, stop=True, tile_position=(64, 0),
                    )
                    at = atp.tile([P, 1024], bf16, tag="at")
                    src_v = sT[:].rearrange("p (h c) -> p h c", h=2)
                    dst_v = at[:].rearrange("p (h c) -> p h c", h=2)
                    nc.scalar.activation(
                        dst_v[:, :, 0:w], src_v[:, :, 0:w],
                        EXP, bias=0.0, scale=0.125,
                    )
                    if i >= 0:
                        # triangle mask on the first live 128 columns
                        for h0 in (0, 512):
                            nc.vector.tensor_mul(
                                at[:, h0 : h0 + P],
                                at[:, h0 : h0 + P], mask_tri[:],
                            )
                    nc.tensor.matmul(
                        ouA[:, c0:W], vv[j][:, 130 * mp : 130 * mp + 65],
                        at[:, 0:w],
                        start=(j == 0), stop=(j == nj - 1),
                    )
                    nc.tensor.matmul(
                        ouB[:, c0:W], vv[j][:, 130 * mp + 65 : 130 * mp + 130],
                        at[:, 512 : 512 + w],
                        start=(j == 0), stop=(j == nj - 1),
                    )
                    step[0] += 1
                    if fillers and step[0] % cadence == 0:
                        fillers.pop(0)()
                # normalization: denominator (psum row 64) reciprocal,
                # broadcast, scale. B half normalized at partitions 0:64 then
                # DMA-shifted to outN partitions 64:128.
                dA = nrm.tile([1, W], f32, tag="dA")
                dB = nrm.tile([1, W], f32, tag="dB")
                nc.any.tensor_copy(dA[:], ouA[64:65, :])
                nc.any.tensor_copy(dB[:], ouB[64:65, :])
                dRA = nrm.tile([1, W], f32, tag="dRA")
                dRB = nrm.tile([1, W], f32, tag="dRB")
                nc.vector.reciprocal_approx_fast(dRA[:], dA[:])
                nc.vector.reciprocal_approx_fast(dRB[:], dB[:])
                cA = nrm.tile([1, W], bf16, tag="cA")
                cB = nrm.tile([1, W], bf16, tag="cB")
                nc.vector.tensor_copy(cA[:], dRA[:])
                nc.vector.tensor_copy(cB[:], dRB[:])
                bcA = nrm.tile([64, W], bf16, tag="bcA")
                bcB = nrm.tile([P, W], bf16, tag="bcB")
                nc.gpsimd.partition_broadcast(bcA[:, :], cA[:], channels=64)
                nc.gpsimd.partition_broadcast(bcB[:, :], cB[:], channels=P)
                # evict psum promptly (frees the AV banks for the next head
                # pair); B half is DMA'd raw to partitions 64:128 early and
                # normalized in place (keeps the shift DMA off the chain tail)
                tbA = nrm.tile([64, W], bf16, tag="tbA")
                tbB = nrm.tile([64, W], bf16, tag="tbB")
                nc.any.tensor_copy(tbA[:], ouA[0:64, :])
                nc.any.tensor_copy(tbB[:], ouB[0:64, :])
                nc.sync.dma_start(outN[mp][64:128, :], tbB[:])
                nc.vector.tensor_mul(outN[mp][0:64, :], tbA[:], bcA[:, :])
                nc.vector.tensor_mul(
                    outN[mp][64:128, :], outN[mp][64:128, :], bcB[64:128, :]
                )
            while fillers:
                fillers.pop(0)()
            return outN

        def rs_chunk(c):
            if globals().get("_NO_COLLECTIVE"):
                nc.sync.dma_start(out_d.ap()[c * P : (c + 1) * P, :], rs_in[c][0:P, :])
            else:
                nc.gpsimd.collective_compute(
                    "ReduceScatter", mybir.AluOpType.add,
                    replica_groups=[[0, 1], [2, 3], [4, 5], [6, 7]],
                    ins=[rs_in[c].opt()], outs=[rs_out[c].opt()],
                )
                nc.sync.dma_start(out_d.ap()[c * P : (c + 1) * P, :], rs_out[c][:])

        def proj(Q0, W, outN, i2s=None):
            for i2 in (range(W // P) if i2s is None else i2s):
                c = Q0 // 256 + i2 // 2  # 256-row RS chunk index
                r = (Q0 // P + i2) % 2   # 128-row position within the chunk
                for n in range(2):
                    ps = spp.tile([P, 512], f32, tag="sp", name=f"fpq{Q0}i{i2}n{n}")
                    for hp in range(4):
                        nc.tensor.matmul(
                            ps[:],
                            outN[hp][:, i2 * P : (i2 + 1) * P],
                            wproj_t[hp][:, n * 512 : (n + 1) * 512],
                            start=(hp == 0), stop=(hp == 3),
                        )
                    fin = finp.tile([P, 512], bf16, tag="fin")
                    nc.vector.tensor_add(fin[:], ps[:], beta_b[:, n * 512 : (n + 1) * 512])
                    nc.sync.dma_start(
                        rs_in[c][r * P : (r + 1) * P, n * 512 : (n + 1) * 512],
                        fin[:],
                    )
                if r == 1:
                    rs_chunk(c)

        # ---------------- main pipeline ----------------
        vproj(0)
        qkproj(0)
        pending = None
        blocks = [(0, 512), (512, 512), (1024, 512), (1536, 512)]
        for bi, (Q0, W) in enumerate(blocks):
            fillers = []
            if pending is not None:
                Qp, Wp, outNp = pending
                for i2 in range(Wp // P):
                    fillers.append(
                        lambda Qp=Qp, Wp=Wp, o=outNp, i2=i2: proj(Qp, Wp, o, [i2])
                    )
            if bi < 3:
                for il in range(4):
                    fillers.append(lambda il=il, q=bi + 1: vproj(q, [il]))
                for m in range(8):
                    fillers.append(lambda m=m, q=bi + 1: qkproj(q, [m]))
            outN = attn(Q0, W, fillers)
            pending = (Q0, W, outN)
        proj(pending[0], pending[1], pending[2])
        _dbg(nc, "qkT0", qkT[0][:])
        _dbg(nc, "qkT4", qkT[4][:])
        _dbg(nc, "vv0", vv[0][:])
        _dbg(nc, "xT0", xT[0][:])


def _build():
    if "nc" in _CACHE:
        return _CACHE["nc"]
    nc = bacc.Bacc("TRN2", target_bir_lowering=False, debug=False, num_devices=NCORES)
    xt_d = nc.dram_tensor("x_t", [D, T], bf16, kind="ExternalInput")
    wqk_d = nc.dram_tensor("w_qk", [D, 1024], bf16, kind="ExternalInput")
    wv_d = nc.dram_tensor("w_v", [D, 512], bf16, kind="ExternalInput")
    bqk_d = nc.dram_tensor("b_qk", [8, P, 1], f32, kind="ExternalInput")
    wproj_d = nc.dram_tensor("w_proj", [512, D], bf16, kind="ExternalInput")
    beta_d = nc.dram_tensor("beta", [1, D], bf16, kind="ExternalInput")
    out_d = nc.dram_tensor("out", [T // 2, D], bf16, kind="ExternalOutput")
    with tile.TileContext(nc) as tc:
        _emit(nc, tc, xt_d, wqk_d, wv_d, bqk_d, wproj_d, beta_d, out_d)
    nc.compile()
    _CACHE["nc"] = nc
    return nc


def make_in_maps(x, w_qkv, b_qkv, w_proj, b_proj):
    x = np.asarray(x, np.float32)
    w_qkv = np.asarray(w_qkv, np.float32)
    b_qkv = np.asarray(b_qkv, np.float32)
    w_proj = np.asarray(w_proj, np.float32)
    b_proj = np.asarray(b_proj, np.float32)
    in_maps = []
    for c in range(NCORES):
        b, g = c // 2, c % 2
        qcols = slice(g * 512, (g + 1) * 512)
        kcols = slice(D + g * 512, D + (g + 1) * 512)
        vcols = slice(2 * D + g * 512, 2 * D + (g + 1) * 512)
        w_qk = np.concatenate([w_qkv[:, qcols], w_qkv[:, kcols]], axis=1)
        b_qk = np.concatenate([b_qkv[qcols], b_qkv[kcols]])
        wp = np.ascontiguousarray(w_proj[g * 512 : (g + 1) * 512, :])
        beta = wp.T @ b_qkv[vcols]
        if g == 0:
            beta = beta + b_proj
        in_maps.append({
            "x_t": np.ascontiguousarray(x[b].T).astype(ml_dtypes.bfloat16),
            "w_qk": np.ascontiguousarray(w_qk).astype(ml_dtypes.bfloat16),
            "w_v": np.ascontiguousarray(w_qkv[:, vcols]).astype(ml_dtypes.bfloat16),
            "b_qk": b_qk.reshape(8, P, 1),
            "w_proj": wp.astype(ml_dtypes.bfloat16),
            "beta": beta.reshape(1, D).astype(ml_dtypes.bfloat16),
        })
    return in_maps


def kernel(x, w_qkv, b_qkv, w_proj, b_proj, trace=False, **run_kwargs):
    global LAST_RESULTS
    nc = _build()
    in_maps = make_in_maps(x, w_qkv, b_qkv, w_proj, b_proj)
    res = run_bass_kernel_spmd(
        nc, in_maps, core_ids=list(range(NCORES)), trace=trace, **run_kwargs
    )
    LAST_RESULTS = res
    out = np.empty((B, T, D), np.float32)
    for b in range(B):
        ev = np.asarray(res.results[2 * b]["out"], dtype=np.float32)
        od = np.asarray(res.results[2 * b + 1]["out"], dtype=np.float32)
        for c in range(8):
            out[b, c * 256 : c * 256 + 128] = ev[c * 128 : (c + 1) * 128]
            out[b, c * 256 + 128 : (c + 1) * 256] = od[c * 128 : (c + 1) * 128]
    return out


# revision 41
# speedup vs baseline: 1.0443x; 1.0031x over previous
"""Causal self-attention Bass kernel for 8 trn2 NeuronCores.

Problem: B=4, T=2048, D=1024, H=16 causal self-attention (qkv proj + attn + out proj).

Sharding: core c = 2*b + g handles batch b (=c//2) and head-group g (=c%2, 8 heads).

Structure (J-pipelined, chunked collective):
  - x is transposed + bf16-cast on the HOST: xT [D, T] arrives ready for matmuls.
  - V projection per t-quarter: vv[i] [keys=128, 4mp x (65A|65B)] bf16, ones col
    at slot 64 of each 65-block (softmax denominator via the AV matmul).
  - QK projection per T-quarter n: qkT[m] [128 dims, T] bf16 (+bias).
  - Attention per query block J (512 q) x head pair mp: transposed-score flash
    loop over key blocks j<=diag; exp on ACT -> at bf16; causal masks on DVE;
    AV accumulates [65, 512] psum (row 64 = denominator).
  - Normalization per (mp, J): reciprocal_approx_fast on the denominator row,
    partition_broadcast, muls; B-half is normalized pre-shift then DMA'd to
    partitions 64:128.
  - Output projection per J -> rs_in[J] bf16 [512, 1024]; pairwise
    ReduceScatter per J (overlaps attention of J+1); even core gets rows
    [0:256) of each 512-row chunk, odd gets [256:512).
Host reassembles interleaved 256-row chunks per batch.
"""

from contextlib import ExitStack

import ml_dtypes
import numpy as np

import concourse.bass as bass
import concourse.mybir as mybir
import concourse.tile as tile
from concourse import bacc
from concourse.bass_utils import run_bass_kernel_spmd

B, T, D, H = 4, 2048, 1024, 16
HD = D // H  # 64
NCORES = 8
P = 128
f32 = mybir.dt.float32
f32r = mybir.dt.float32r
bf16 = mybir.dt.bfloat16
EXP = mybir.ActivationFunctionType.Exp

_CACHE = {}
LAST_RESULTS = None
_DEBUG_SINK = None


def _dbg(nc, name, ap):
    if _DEBUG_SINK is not None and name in _DEBUG_SINK:
        nc.sync.dma_start(_DEBUG_SINK[name].ap(), ap)


def _emit(nc, tc, xt_d, wqk_d, wv_d, bqk_d, wproj_d, beta_d, out_d):
    with ExitStack() as ctx:
        # ---------------- constants / persistent tiles ----------------
        const = ctx.enter_context(tc.tile_pool(name="const", bufs=1))
        mask_tri = const.tile([P, P], bf16, tag="mask_tri")
        nc.gpsimd.memset(mask_tri[:], 1.0)
        nc.gpsimd.affine_select(
            out=mask_tri[:], in_=mask_tri[:],
            compare_op=mybir.AluOpType.is_ge, fill=0.0,
            base=0, pattern=[[1, P]], channel_multiplier=-1,
        )
        zeros384 = const.tile([P, 384], bf16, tag="zeros384")
        nc.vector.memset(zeros384[:], 0.0)
        bq = [const.tile([P, 1], f32, tag=f"bq{m}", name=f"bq{m}") for m in range(8)]
        beta_b = const.tile([P, D], bf16, tag="beta_b")

        # persistent activations
        xt_pool = ctx.enter_context(tc.tile_pool(name="xt", bufs=1))
        xT = [xt_pool.tile([P, T], bf16, tag=f"xT{k}", name=f"xT{k}") for k in range(8)]
        qkt_pool = ctx.enter_context(tc.tile_pool(name="qkt", bufs=1))
        qkT = [qkt_pool.tile([P, T], bf16, tag=f"qkT{m}", name=f"qkT{m}") for m in range(8)]
        vv_pool = ctx.enter_context(tc.tile_pool(name="vv", bufs=1))
        vv = [vv_pool.tile([P, 520], bf16, tag=f"vv{i}", name=f"vv{i}") for i in range(16)]

        # weights
        wp = ctx.enter_context(tc.tile_pool(name="wts", bufs=1))
        wqk_t = [wp.tile([P, 1024], bf16, tag=f"wqk{k}", name=f"wqk{k}") for k in range(8)]
        wv_t = [wp.tile([P, 512], bf16, tag=f"wv{k}", name=f"wv{k}") for k in range(8)]
        wproj_t = [wp.tile([P, D], bf16, tag=f"wp{hp}", name=f"wp{hp}") for hp in range(4)]

        # ones columns of vv (denominator trick): col 64 of each 65-block
        ones8 = const.tile([P, 8], bf16, tag="ones8")
        nc.vector.memset(ones8[:], 1.0)
        ones_src = ones8[:].rearrange("p (mp h one) -> p mp h one", mp=4, h=2)
        for i in range(16):
            dst = vv[i][:].rearrange("p (mp h d) -> p mp h d", mp=4, h=2)
            nc.vector.tensor_copy(dst[:, :, :, 64:65], ones_src[:, :, :, :])

        # working pools
        onp = ctx.enter_context(tc.tile_pool(name="outn", bufs=3))
        atp = ctx.enter_context(tc.tile_pool(name="atp", bufs=4))
        nrm = ctx.enter_context(tc.tile_pool(name="nrm", bufs=2))
        finp = ctx.enter_context(tc.tile_pool(name="finp", bufs=3))

        spp = ctx.enter_context(tc.tile_pool(name="spp", bufs=2, space="PSUM"))
        stps = ctx.enter_context(tc.tile_pool(name="stps", bufs=2, space="PSUM"))
        oups = ctx.enter_context(tc.tile_pool(name="oups", bufs=1, space="PSUM"))

        dram = ctx.enter_context(tc.tile_pool(name="dram", bufs=1, space="DRAM"))
        # one RS chunk per 256-row half of each query block J (8 total):
        # separate tiles so each collective depends only on its own writes
        rs_in = [dram.tile([256, D], bf16, tag=f"rsi{c}", name=f"rsi{c}") for c in range(8)]
        rs_out = [dram.tile([128, D], bf16, tag=f"rso{c}", name=f"rso{c}") for c in range(8)]

        # ~4us of dummy matmuls at the head of the PE queue: flips the HAM
        # clock gate to 8/8 while the first input DMAs are still in flight,
        # so the real matmuls start at 2.4 GHz.
        wps = spp.tile([P, 384], f32, tag="sp", name="warm")
        for d in range(24):
            nc.tensor.matmul(
                wps[:], zeros384[:, 0:128], zeros384[:],
                start=(d == 0), stop=(d == 23),
            )

        # ---------------- DMAs: quarter 0 + weights first, 3 queues ----------------
        for k in range(8):
            nc.sync.dma_start(xT[k][:, 0:512], xt_d.ap()[k * P : (k + 1) * P, 0:512])
        for k in range(8):
            nc.scalar.dma_start(wv_t[k][:], wv_d.ap()[k * P : (k + 1) * P, :])
        for k in range(8):
            nc.gpsimd.dma_start(wqk_t[k][:], wqk_d.ap()[k * P : (k + 1) * P, :])
        for m in range(8):
            nc.gpsimd.dma_start(bq[m][:], bqk_d.ap()[m])
        for q in range(1, 4):
            for k in range(8):
                nc.sync.dma_start(
                    xT[k][:, q * 512 : (q + 1) * 512],
                    xt_d.ap()[k * P : (k + 1) * P, q * 512 : (q + 1) * 512],
                )
        for hp in range(4):
            nc.gpsimd.dma_start(wproj_t[hp][:], wproj_d.ap()[hp * P : (hp + 1) * P, :])
        nc.gpsimd.dma_start(beta_b[0:1, :], beta_d.ap())
        nc.gpsimd.partition_broadcast(beta_b[:], beta_b[0:1, :], channels=P)

        def vproj(q, ils=range(4)):
            # v for t-tiles of quarter q: vv[i] [keys=128, (mp h 65)]
            for il in ils:
                i = q * 4 + il
                ps = spp.tile([P, 512], f32, tag="sp", name=f"vp{i}")
                for k in range(8):
                    nc.tensor.matmul(
                        ps[:],
                        xT[k][:, i * P : (i + 1) * P],
                        wv_t[k][:],
                        start=(k == 0), stop=(k == 7),
                    )
                src = ps[:].rearrange("p (mp h d) -> p mp h d", mp=4, h=2)
                dst = vv[i][:].rearrange("p (mp h d) -> p mp h d", mp=4, h=2)
                nc.vector.tensor_copy(dst[:, :, :, 0:64], src[:, :, :, :])

        def qkproj(n, ms=range(8)):
            ns = slice(n * 512, (n + 1) * 512)
            for m in ms:
                ps = spp.tile([P, 512], f32, tag="sp", name=f"qkp{m}n{n}")
                for k in range(8):
                    nc.tensor.matmul(
                        ps[:],
                        wqk_t[k][:, m * P : (m + 1) * P],
                        xT[k][:, ns],
                        start=(k == 0), stop=(k == 7),
                    )
                nc.vector.tensor_scalar_add(qkT[m][:, ns], ps[:], bq[m][:])

        def attn(Q0, W, fillers=()):
            # attention for queries [Q0, Q0+W) across all 4 head pairs.
            # W in {256, 512}; key blocks 0..nj-1 cover [0, Q0+W).
            nj = (Q0 + W) // P
            jq = Q0 // P  # first diagonal-straddling key block
            fillers = list(fillers)
            fine = Q0 == 0  # short exp-bound block: consume fillers per-j
            outN = [onp.tile([P, W], bf16, tag=f"outN{mp}", name=f"outN{mp}Q{Q0}")
                    for mp in range(4)]
            for mp in range(4):
                qs, ks = qkT[mp], qkT[4 + mp]
                ouA = oups.tile([65, W], f32, tag="ouA")
                ouB = oups.tile([65, W], f32, tag="ouB")
                for j in range(nj):
                    # the two head halves always sit at column offsets 0 and
                    # 512 (separate PSUM banks — the concurrent row-group
                    # matmuls must not share a bank)
                    sT = stps.tile([P, 1024], f32, tag="sT")
                    js = slice(j * P, (j + 1) * P)
                    i = j - jq
                    # diagonal-straddling blocks: queries < c0 are fully
                    # masked, so scores/exp/AV all restrict to [c0:W)
                    c0 = P * i if i > 0 else 0
                    qJs = slice(Q0 + c0, Q0 + W)
                    w = W - c0
                    nc.tensor.matmul(
                        sT[:, 0:w], ks[0:64, js], qs[0:64, qJs],
                        start=True, stop=True, tile_position=(0, 0),
                    )
                    nc.tensor.matmul(
                        sT[:, 512 : 512 + w], ks[64:128, js], qs[64:128, qJs],
                        start=True, stop=True, tile_position=(64, 0),
                    )
                    at = atp.tile([P, 1024], bf16, tag="at")
                    src_v = sT[:].rearrange("p (h c) -> p h c", h=2)
                    dst_v = at[:].rearrange("p (h c) -> p h c", h=2)
                    nc.scalar.activation(
                        dst_v[:, :, 0:w], src_v[:, :, 0:w],
                        EXP, bias=0.0, scale=0.125,
                    )
                    if i >= 0:
                        # triangle mask on the first live 128 columns
                        for h0 in (0, 512):
                            nc.vector.tensor_mul(
                                at[:, h0 : h0 + P],
                                at[:, h0 : h0 + P], mask_tri[:],
                            )
                    nc.tensor.matmul(
                        ouA[:, c0:W], vv[j][:, 130 * mp : 130 * mp + 65],
                        at[:, 0:w],
                        start=(j == 0), stop=(j == nj - 1),
                    )
                    nc.tensor.matmul(
                        ouB[:, c0:W], vv[j][:, 130 * mp + 65 : 130 * mp + 130],
                        at[:, 512 : 512 + w],
                        start=(j == 0), stop=(j == nj - 1),
                    )
                    if fine and fillers:
                        fillers.pop(0)()
                # normalization: denominator (psum row 64) reciprocal,
                # broadcast, scale. B half normalized at partitions 0:64 then
                # DMA-shifted to outN partitions 64:128.
                dA = nrm.tile([1, W], f32, tag="dA")
                dB = nrm.tile([1, W], f32, tag="dB")
                nc.any.tensor_copy(dA[:], ouA[64:65, :])
                nc.any.tensor_copy(dB[:], ouB[64:65, :])
                dRA = nrm.tile([1, W], f32, tag="dRA")
                dRB = nrm.tile([1, W], f32, tag="dRB")
                nc.vector.reciprocal_approx_fast(dRA[:], dA[:])
                nc.vector.reciprocal_approx_fast(dRB[:], dB[:])
                cA = nrm.tile([1, W], bf16, tag="cA")
                cB = nrm.tile([1, W], bf16, tag="cB")
                nc.vector.tensor_copy(cA[:], dRA[:])
                nc.vector.tensor_copy(cB[:], dRB[:])
                bcA = nrm.tile([64, W], bf16, tag="bcA")
                bcB = nrm.tile([P, W], bf16, tag="bcB")
                nc.gpsimd.partition_broadcast(bcA[:, :], cA[:], channels=64)
                nc.gpsimd.partition_broadcast(bcB[:, :], cB[:], channels=P)
                # evict psum promptly (frees the AV banks for the next head
                # pair); B half is DMA'd raw to partitions 64:128 early and
                # normalized in place (keeps the shift DMA off the chain tail)
                tbA = nrm.tile([64, W], bf16, tag="tbA")
                tbB = nrm.tile([64, W], bf16, tag="tbB")
                nc.any.tensor_copy(tbA[:], ouA[0:64, :])
                nc.any.tensor_copy(tbB[:], ouB[0:64, :])
                nc.sync.dma_start(outN[mp][64:128, :], tbB[:])
                nc.vector.tensor_mul(outN[mp][0:64, :], tbA[:], bcA[:, :])
                nc.vector.tensor_mul(
                    outN[mp][64:128, :], outN[mp][64:128, :], bcB[64:128, :]
                )
                # interleave independent fill work (prev proj / next
                # projections) so the in-order PE queue never starves on the
                # normalization chain or exp latency
                if not fine and fillers:
                    fillers.pop(0)()
            while fillers:
                fillers.pop(0)()
            return outN

        def rs_chunk(c):
            if globals().get("_NO_COLLECTIVE"):
                nc.sync.dma_start(out_d.ap()[c * P : (c + 1) * P, :], rs_in[c][0:P, :])
            else:
                nc.gpsimd.collective_compute(
                    "ReduceScatter", mybir.AluOpType.add,
                    replica_groups=[[0, 1], [2, 3], [4, 5], [6, 7]],
                    ins=[rs_in[c].opt()], outs=[rs_out[c].opt()],
                )
                nc.sync.dma_start(out_d.ap()[c * P : (c + 1) * P, :], rs_out[c][:])

        def proj(Q0, W, outN, i2s=None):
            for i2 in (range(W // P) if i2s is None else i2s):
                c = Q0 // 256 + i2 // 2  # 256-row RS chunk index
                r = (Q0 // P + i2) % 2   # 128-row position within the chunk
                for n in range(2):
                    ps = spp.tile([P, 512], f32, tag="sp", name=f"fpq{Q0}i{i2}n{n}")
                    for hp in range(4):
                        nc.tensor.matmul(
                            ps[:],
                            outN[hp][:, i2 * P : (i2 + 1) * P],
                            wproj_t[hp][:, n * 512 : (n + 1) * 512],
                            start=(hp == 0), stop=(hp == 3),
                        )
                    fin = finp.tile([P, 512], bf16, tag="fin")
                    nc.vector.tensor_add(fin[:], ps[:], beta_b[:, n * 512 : (n + 1) * 512])
                    nc.sync.dma_start(
                        rs_in[c][r * P : (r + 1) * P, n * 512 : (n + 1) * 512],
                        fin[:],
                    )
                if r == 1:
                    rs_chunk(c)

        # ---------------- main pipeline ----------------
        vproj(0)
        qkproj(0)
        pending = None
        blocks = [(0, 512), (512, 512), (1024, 512), (1536, 512)]
        for bi, (Q0, W) in enumerate(blocks):
            fillers = []
            if pending is not None:
                Qp, Wp, outNp = pending
                fillers.append(lambda Qp=Qp, Wp=Wp, o=outNp: proj(Qp, Wp, o))
            if bi == 0:
                # fine-grained fillers: consumed per key-block in attn(0),
                # which is short and exp-latency-bound
                for il in range(4):
                    fillers.append(lambda il=il: vproj(1, [il]))
                for m in range(8):
                    fillers.append(lambda m=m: qkproj(1, [m]))
            elif bi < 3:
                fillers.append(lambda q=bi + 1: vproj(q))
                fillers.append(lambda q=bi + 1: qkproj(q, range(4)))
                fillers.append(lambda q=bi + 1: qkproj(q, range(4, 8)))
            outN = attn(Q0, W, fillers)
            pending = (Q0, W, outN)
        proj(pending[0], pending[1], pending[2])
        _dbg(nc, "qkT0", qkT[0][:])
        _dbg(nc, "qkT4", qkT[4][:])
        _dbg(nc, "vv0", vv[0][:])
        _dbg(nc, "xT0", xT[0][:])


def _build():
    if "nc" in _CACHE:
        return _CACHE["nc"]
    nc = bacc.Bacc("TRN2", target_bir_lowering=False, debug=False, num_devices=NCORES)
    xt_d = nc.dram_tensor("x_t", [D, T], bf16, kind="ExternalInput")
    wqk_d = nc.dram_tensor("w_qk", [D, 1024], bf16, kind="ExternalInput")
    wv_d = nc.dram_tensor("w_v", [D, 512], bf16, kind="ExternalInput")
    bqk_d = nc.dram_tensor("b_qk", [8, P, 1], f32, kind="ExternalInput")
    wproj_d = nc.dram_tensor("w_proj", [512, D], bf16, kind="ExternalInput")
    beta_d = nc.dram_tensor("beta", [1, D], bf16, kind="ExternalInput")
    out_d = nc.dram_tensor("out", [T // 2, D], bf16, kind="ExternalOutput")
    with tile.TileContext(nc) as tc:
        _emit(nc, tc, xt_d, wqk_d, wv_d, bqk_d, wproj_d, beta_d, out_d)
    nc.compile()
    _CACHE["nc"] = nc
    return nc


def make_in_maps(x, w_qkv, b_qkv, w_proj, b_proj):
    x = np.asarray(x, np.float32)
    w_qkv = np.asarray(w_qkv, np.float32)
    b_qkv = np.asarray(b_qkv, np.float32)
    w_proj = np.asarray(w_proj, np.float32)
    b_proj = np.asarray(b_proj, np.float32)
    in_maps = []
    for c in range(NCORES):
        b, g = c // 2, c % 2
        qcols = slice(g * 512, (g + 1) * 512)
        kcols = slice(D + g * 512, D + (g + 1) * 512)
        vcols = slice(2 * D + g * 512, 2 * D + (g + 1) * 512)
        w_qk = np.concatenate([w_qkv[:, qcols], w_qkv[:, kcols]], axis=1)
        b_qk = np.concatenate([b_qkv[qcols], b_qkv[kcols]])
        wp = np.ascontiguousarray(w_proj[g * 512 : (g + 1) * 512, :])
        beta = wp.T @ b_qkv[vcols]
        if g == 0:
            beta = beta + b_proj
        in_maps.append({
            "x_t": np.ascontiguousarray(x[b].T).astype(ml_dtypes.bfloat16),
            "w_qk": np.ascontiguousarray(w_qk).astype(ml_dtypes.bfloat16),
            "w_v": np.ascontiguousarray(w_qkv[:, vcols]).astype(ml_dtypes.bfloat16),
            "b_qk": b_qk.reshape(8, P, 1),
            "w_proj": wp.astype(ml_dtypes.bfloat16),
            "beta": beta.reshape(1, D).astype(ml_dtypes.bfloat16),
        })
    return in_maps


def kernel(x, w_qkv, b_qkv, w_proj, b_proj, trace=False, **run_kwargs):
    global LAST_RESULTS
    nc = _build()
    in_maps = make_in_maps(x, w_qkv, b_qkv, w_proj, b_proj)
    res = run_bass_kernel_spmd(
        nc, in_maps, core_ids=list(range(NCORES)), trace=trace, **run_kwargs
    )
    LAST_RESULTS = res
    out = np.empty((B, T, D), np.float32)
    for b in range(B):
        ev = np.asarray(res.results[2 * b]["out"], dtype=np.float32)
        od = np.asarray(res.results[2 * b + 1]["out"], dtype=np.float32)
        for c in range(8):
            out[b, c * 256 : c * 256 + 128] = ev[c * 128 : (c + 1) * 128]
            out[b, c * 256 + 128 : (c + 1) * 256] = od[c * 128 : (c + 1) * 128]
    return out
